# revision 13
# baseline (speedup 1.0000x reference)
"""Trainium2 Bass kernel for nn_AdaptiveSNN (B=128, T=32, D=6400, H=1000, A=4).

Strategy (data-parallel over batch, 8 NeuronCores, 16 batch rows each):

  The heavy layer-1 matmul h1[b,t,:] = x[b,t,:] @ W1.T is NOT sequential in t
  (the LIF recurrence only couples the cheap elementwise state update), so per
  core we compute H1 = X_local @ W1.T as one [512, 6400] x [6400, 1024] matmul
  (H padded 1000->1024), laid out transposed: psum banks hold H1.T chunks
  [128 H, 512 cols] with col = t*16 + b (t-major, so per-step LIF slices are
  contiguous 16-element runs and layer-2 column ranges by time are contiguous).

  fp16 hi/lo x3 matmul: fp32 operands are split a = ah + al with ah = fp16(a),
  al = fp16((a - ah) * 2^12); the product needs ah*bh (psum bank HI) and
  ah*bl + al*bh (psum bank LO, uniformly scaled 2^12); al*bl (~2^-24 relative)
  is dropped.  h = HI + 2^-12 * LO then matches an fp32 matmul up to normal
  fp32 rounding (fp16 products are exact in fp32; PSUM accumulates fp32).
  W1 is pre-scaled by 256 so its lo-part stays in fp16 normal range; the LIF
  recurrence is scale-invariant, so mem1 simply runs at 256x with threshold
  256 (exact powers of two).  fp16 streams 1 cycle/row through the PE vs ~6
  effective for fp32.

  - lhsT = W1.T tiles (host pre-transposed), rhs = X.T tiles (host
    pre-transposed), K = D on partitions, 50 k-tiles of 128.
  - m-outer loop over 8 H-chunks; the first LIF group's chunks are
    k-interleaved so the 13MB x load spreads over the first sweep.  W1
    streams on the Sync HWDGE queue, x and constants on the Scalar HWDGE
    queue (two independent FIFOs).
  - x chunks are split into fine pieces so consumers wait per-piece, not for
    a whole 1.3MB transfer (a monolithic chunk 1 cost a 4.9us PE stall).
  - LIF scan: 3 scalar_tensor_tensor ops per step (all hit the DVE 2x_2p
    fast path, ~115ns each, vs tensor_tensor/tensor_scalar at ~205ns):
      acc_t  = beta*mem + h_t                      (mult, add)
      keep_t = (acc_t <= thr) is_ge keep_{t-1}     (is_le, is_ge)  [exact:
               keep_prev=0 -> mem_t=0 -> keep_t=1 = x is_ge 0]
      mem_t  = (acc_t * 1) * keep_{t-1}            (mult, mult)
    keep_t is independent of mem_t, so mm2 unblocks one op earlier.
  - evac h = HI + 2^-12*LO as a single stt reading both psum banks.
  - Layer 2: h2 = spk1 @ W2.T + b2 = (sum(W2)+b2) - keep1 @ W2.T, accumulated
    per chunk as fp16 hi/lo matmuls into one psum bank per time-half (hi sums
    in cols [0,256), 2^12-scaled lo sums in [256,512); one full-width
    start=True opener per bank, emitted during the startup DMA window).
  - The last H-chunk (m7) runs in two column(=time) phases (18/14 steps;
    224 cols is the smallest matmul-bound sweep width); phase B is pipelined
    in sub-chunks: DVE scans a few LIF1 steps, the PE accumulates the mm2
    slice for those columns, DVE runs LIF2 and the output DMA goes out,
    shortening the exposed serial tail.
  - Output is keep2 as [A, 512]; host computes spk2 = 1 - keep2 and
    transposes back.  Each group's layer-2 matmuls are emitted mid-way into
    the next sweep so the PE reaches them after keep is ready.

  (fp32r was measured at ~1e-3 error on HW; with only ~300 output spikes a
  single threshold flip fails the rel-err gate, so only fp32-grade math is
  usable: the fp64 margin analysis shows layer-1 decision margins down to
  6e-6.  This fp16x3 kernel is bit-identical to the jax fp32 reference
  output on the benchmark inputs.)
"""

import sys
import types

import numpy as np

# bass_utils imports antenv.axon_hooks when BASS_TRACE is set; the module is
# absent in some images -- degrade to no tracing instead of crashing.
try:
    import antenv.axon_hooks  # noqa: F401
except ImportError:
    _m = types.ModuleType("antenv.axon_hooks")
    _m.get_axon_ntff_profile_hook = lambda: None
    _m.set_axon_ntff_profile_hook = lambda h: None
    sys.modules["antenv.axon_hooks"] = _m

import concourse.bass as bass
import concourse.tile as tile
from concourse import bacc, mybir
from concourse.bass_utils import run_bass_kernel_spmd

F32 = mybir.dt.float32
F16 = mybir.dt.float16
OP = mybir.AluOpType
AF = mybir.ActivationFunctionType

NCORES = 8
B, T, D, H, A = 128, 32, 6400, 1000, 4
BL = B // NCORES            # 16 local batch
COLS = BL * T               # 512 matmul columns, col = t*16 + b (t-major)
KT = D // 128               # 50 k tiles
HP = 1024                   # padded H
M = HP // 128               # 8 H-chunks
BETA = 1.0 - 0.01

WSCALE = 256.0              # W1 pre-scale (exact power of 2)
LSCALE = 4096.0             # lo-part scale 2^12

# FP16X3 True: hi/lo fp16 3-pass matmul.  False: plain fp32 matmul.
FP16X3 = True

# scheduling feature flags (validated by microbench + HW runs)
STT_SCAN = True             # 3x scalar_tensor_tensor LIF step
# NOTE: a single stt reading both psum banks is ILLEGAL (NCC_IBVF027: only
# one non-scalar input may come from PSUM) -- evac splits across Act + DVE.
EVAC_FUSED = False
# NOTE: Pool rejects TensorScalarPtr outright (NCC_IXCG966) -- LIF2 on DVE.
LIF2_ON_POOL = False
SUBCHUNK_TAIL = True        # pipeline m7 phase-B LIF1/mm2/LIF2 in sub-chunks

XCH = 5                     # x DMA chunks (10 k-tiles each)
XKT = KT // XCH
W1H = 2                     # w1 DMA halves per m-chunk (25 k-tiles each)
W1KT = KT // W1H

_CACHE = {}


def _lif_steps(nc, memv, accv, h_at, k_at, thresh, t_range=None, eng=None):
    """Emit the LIF recurrence for timesteps in t_range (default all).

    memv/accv: [p, ..., b] fp32 SBUF views; h_at(t)/k_at(t) return the
    per-step views.  keep column t holds (mem_t <= thresh) = 1 - spk_t.
    """
    eng = eng if eng is not None else nc.vector
    ts_list = list(t_range if t_range is not None else range(T))
    for t in ts_list:
        skip_mem = (t == T - 1)  # mem after the last step is never read
        if t == 0:
            # mem=0, keep=1: mem <- h_0  (beta*0 + h)
            eng.scalar_tensor_tensor(
                out=memv, in0=memv, scalar=BETA,
                in1=h_at(0), op0=OP.mult, op1=OP.add)
            eng.tensor_scalar(
                out=k_at(0), in0=memv, scalar1=thresh,
                scalar2=None, op0=OP.is_le)
        elif STT_SCAN:
            eng.scalar_tensor_tensor(
                out=accv, in0=memv, scalar=BETA,
                in1=h_at(t), op0=OP.mult, op1=OP.add)
            # keep_t = (acc_t <= thr) is_ge keep_{t-1}; exact because
            # keep_{t-1}=0 forces mem_t=0 which always keeps.
            eng.scalar_tensor_tensor(
                out=k_at(t), in0=accv, scalar=thresh,
                in1=k_at(t - 1), op0=OP.is_le, op1=OP.is_ge)
            if not skip_mem:
                eng.scalar_tensor_tensor(
                    out=memv, in0=accv, scalar=1.0,
                    in1=k_at(t - 1), op0=OP.mult, op1=OP.mult)
        else:
            eng.scalar_tensor_tensor(
                out=accv, in0=memv, scalar=BETA,
                in1=h_at(t), op0=OP.mult, op1=OP.add)
            eng.tensor_tensor(
                out=memv, in0=accv, in1=k_at(t - 1), op=OP.mult)
            eng.tensor_scalar(
                out=k_at(t), in0=memv, scalar1=thresh,
                scalar2=None, op0=OP.is_le)


def build(with_b1=True):
    nc = bacc.Bacc("TRN2", target_bir_lowering=False, debug=False,
                   num_devices=NCORES)

    MMDT = F16 if FP16X3 else F32
    THR1 = 1.0 * WSCALE if FP16X3 else 1.0

    # host layouts (see _prep_shared/_prep_x for the exact packing):
    #   xh/xl [128(p), KT, COLS]      x.T tiles, col = t*16+b, hi/lo fp16
    #   w1h/w1l [M, 128(p), KT, 128]  (256*W1).T tiles, hi/lo fp16
    #   b1hl  [1, 2*HP]               256*b1 hi/lo rows
    #   w2x  [128(p), 2*M*A]          -W2p hi/lo blocks, w2[p, m*4+a]
    #   s2x  [1, 3*A]                 [s2h | s2l' | -s2h], s2 = sum(W2p)+b2
    xh_e = nc.declare_dram_parameter("xh", [128, KT, COLS], MMDT, isOutput=False)
    w1h_e = nc.declare_dram_parameter("w1h", [M, 128, KT, 128], MMDT, isOutput=False)
    b1h_e = nc.declare_dram_parameter("b1hl", [1, (2 * HP if FP16X3 else HP)],
                                      MMDT, isOutput=False)
    if FP16X3:
        xl_e = nc.declare_dram_parameter("xl", [128, KT, COLS], F16, isOutput=False)
        w1l_e = nc.declare_dram_parameter("w1l", [M, 128, KT, 128], F16, isOutput=False)
    if FP16X3:
        # negated W2 in hi/lo fp16 (lo scaled 2^12): [hi | lo] blocks
        w2_e = nc.declare_dram_parameter("w2x", [128, 2 * M * A], F16,
                                         isOutput=False)
        # [s2h | s2l' | -s2h] rows for the dual-region opener
        s2_e = nc.declare_dram_parameter("s2x", [1, 3 * A], F16, isOutput=False)
    else:
        w2_e = nc.declare_dram_parameter("w2n", [128, M * A], F32, isOutput=False)
        s2_e = nc.declare_dram_parameter("s2b2", [1, A], F32, isOutput=False)
    out_e = nc.declare_dram_parameter("out", [A, COLS], F32, isOutput=True)

    with tile.TileContext(nc) as tc:
        with (
            tc.tile_pool(name="const", bufs=1) as cpool,
            tc.tile_pool(name="xsb", bufs=(2 * XCH if FP16X3 else XCH)) as xpool,
            tc.tile_pool(name="w1", bufs=(12 if FP16X3 else 4)) as wpool,
            tc.tile_pool(name="h1g", bufs=2) as hpool,
            tc.tile_pool(name="keep", bufs=2) as kpool,
            tc.tile_pool(name="scratch", bufs=2) as spool,
            tc.tile_pool(name="ps1", bufs=(6 if FP16X3 else 7), space="PSUM") as ps1,
            tc.tile_pool(name="ps2", bufs=1, space="PSUM") as ps2,
        ):
            # Small constants + x go on the Scalar HWDGE queue; W1 streams on
            # the Sync HWDGE queue.  Two independent FIFOs -> W1's first tiles
            # aren't stuck behind 13MB of x.
            ones = cpool.tile([1, COLS], MMDT)
            nc.vector.memset(ones, 1.0)
            ones32 = cpool.tile([1, COLS], F32)
            nc.vector.memset(ones32, 1.0)
            # warm the Activation engine's function table (ACT_TABLE_LOAD is
            # ~1.3us once per func) during the initial DMA wait, so the first
            # evac's scaled-copy isn't delayed by it
            actwarm = cpool.tile([1, 8], F32)
            nc.scalar.activation(out=actwarm, in_=ones32[:, :8], func=AF.Copy,
                                 scale=0.5)
            b1h = b1l = None
            if with_b1:
                b1hl = cpool.tile([1, (2 * HP if FP16X3 else HP)], MMDT)
                nc.scalar.dma_start(out=b1hl, in_=b1h_e.ap())
                b1h = b1hl[:, :HP]
                if FP16X3:
                    b1l = b1hl[:, HP:]

            mem1 = cpool.tile([128, M * BL], F32)
            nc.vector.memset(mem1, 0.0)
            mem1v = mem1.rearrange("p (m b) -> p m b", m=M)
            mem2 = cpool.tile([A, BL], F32)
            nc.vector.memset(mem2, 0.0)
            keep2 = cpool.tile([A, COLS], F32)
            h2sb = cpool.tile([A, COLS], F32)
            acc2 = cpool.tile([A, BL], F32)

            # x load in fine pieces: consumers (k-tile matmuls) then wait for
            # their own piece, not a whole chunk.  Chunks 3-4 are DMA'd later
            # on the sync queue (behind group-0's W1): they aren't consumed
            # until ~60% through the first sweep, and deferring them keeps
            # early HBM demand under the limit.
            xparams = [xh_e, xl_e] if FP16X3 else [xh_e]
            xtiles = [[] for _ in xparams]
            deferred_x = []
            for xc in range(XCH):
                xts = [xpool.tile([128, XKT * COLS], MMDT, tag="x", name=f"x{xi}")
                       for xi in range(len(xparams))]
                # hi/lo pieces interleaved so the lo stream is never a full
                # chunk behind the hi stream the PE is consuming; fine pieces
                # mean a k-tile's matmul waits for its own piece, not the
                # whole chunk.
                npieces = 10 if xc == 0 else (5 if xc < 3 else 2)
                edges = [xc * XKT + (XKT * p) // npieces
                         for p in range(npieces + 1)]
                for p in range(npieces):
                    k0, k1 = edges[p], edges[p + 1]
                    o0 = (k0 - xc * XKT) * COLS
                    o1 = (k1 - xc * XKT) * COLS
                    for xi, xe in enumerate(xparams):
                        if xc >= 3:
                            deferred_x.append(
                                (xts[xi][:, o0:o1], xe.ap()[:, k0:k1, :]))
                        else:
                            nc.scalar.dma_start(
                                out=xts[xi][:, o0:o1], in_=xe.ap()[:, k0:k1, :])
                for xi in range(len(xparams)):
                    xtiles[xi].append(xts[xi])

            def x_rhs(xi, k):
                xt = xtiles[xi][k // XKT]
                o = (k % XKT) * COLS
                return xt[:, o:o + COLS]

            # w2/s2 are not needed until the first group finishes
            W2DT = F16 if FP16X3 else F32
            w2sb = cpool.tile([128, (2 * M * A if FP16X3 else M * A)], W2DT)
            nc.scalar.dma_start(out=w2sb, in_=w2_e.ap())
            s2sb = cpool.tile([1, (3 * A if FP16X3 else A)], W2DT)
            nc.scalar.dma_start(out=s2sb, in_=s2_e.ap())
            # One psum bank per time-half.  With fp16 W2 the bank holds two
            # regions: hi sums in cols [0,CH), lo sums (2^12-scaled) in
            # [CH,2CH); a single full-width start=True opener avoids the
            # illegal interleaved-starts-on-one-bank pattern.
            psum2h = [ps2.tile([A, COLS], F32, name=f"p2_{h}", tag=f"p2{h}")
                      for h in range(2)]

            wparams = [w1h_e, w1l_e] if FP16X3 else [w1h_e]
            TH = T // 2
            CH = COLS // 2          # column half, t-major: cols [0,CH) = t<TH

            def stream_w1(ms, hf, finely=False):
                """Stream this k-half of W1 for the chunks in ms, pieces
                interleaved across (chunk, hi/lo) so consumers stay in
                lockstep.  Returns {(chunk_idx, dtype_idx): tile}."""
                tiles = {}
                for i in range(len(ms)):
                    for wi in range(len(wparams)):
                        tiles[(i, wi)] = wpool.tile(
                            [128, W1KT * 128], MMDT, tag="w1", name="w1t")
                nq = 5 if finely else 1
                step = W1KT // nq
                for q in range(nq):
                    for i, m in enumerate(ms):
                        for wi, we in enumerate(wparams):
                            nc.sync.dma_start(
                                out=tiles[(i, wi)][:, q * step * 128:
                                                   (q + 1) * step * 128],
                                in_=we.ap()[m, :, hf * W1KT + q * step:
                                            hf * W1KT + (q + 1) * step, :])
                return tiles

            def k_sweep(ms, phs, pls, cs, finely=False, hooks=None):
                """Bias + 50 k-tile matmuls for the chunks in ms over column
                slice cs, k-interleaved across chunks (spreads the DMA demand
                of the first group over twice the PE time).  hooks is a dict
                {(hf, kk): [callables]} fired at that emission point -- used
                to place the previous group's layer-2 matmuls mid-sweep (so
                the PE reaches them well after their keep operand is ready)
                and to defer x DMA emission."""
                ncols = cs.stop - cs.start
                hooks = hooks or {}
                if with_b1:
                    for i, m in enumerate(ms):
                        nc.tensor.matmul(
                            phs[i], lhsT=b1h[:, m * 128:(m + 1) * 128],
                            rhs=ones[:, :ncols], start=True, stop=False)
                        if FP16X3:
                            nc.tensor.matmul(
                                pls[i], lhsT=b1l[:, m * 128:(m + 1) * 128],
                                rhs=ones[:, :ncols], start=True, stop=False)
                for hf in range(W1H):
                    wts = stream_w1(ms, hf, finely=(finely and hf == 0))
                    for kk in range(W1KT):
                        for hook in hooks.get((hf, kk), ()):
                            hook()
                        k = hf * W1KT + kk
                        start = (not with_b1) and k == 0
                        last = (k == KT - 1)
                        sl = slice(kk * 128, (kk + 1) * 128)
                        # hi*hi -> HI bank; hi*lo + lo*hi -> LO bank
                        for i in range(len(ms)):
                            nc.tensor.matmul(
                                phs[i], lhsT=wts[(i, 0)][:, sl],
                                rhs=x_rhs(0, k)[:, cs],
                                start=start, stop=last)
                            if FP16X3:
                                nc.tensor.matmul(
                                    pls[i], lhsT=wts[(i, 0)][:, sl],
                                    rhs=x_rhs(1, k)[:, cs],
                                    start=start, stop=False)
                                nc.tensor.matmul(
                                    pls[i], lhsT=wts[(i, 1)][:, sl],
                                    rhs=x_rhs(0, k)[:, cs],
                                    start=False, stop=last)

            def evac(hslc, ph, pl):
                # h = HI + 2^-12 * LO  (h stays at 256*h1 scale)
                if not FP16X3:
                    nc.vector.tensor_copy(hslc, ph)
                elif EVAC_FUSED:
                    nc.vector.scalar_tensor_tensor(
                        out=hslc, in0=pl, scalar=1.0 / LSCALE, in1=ph,
                        op0=OP.mult, op1=OP.add)
                else:
                    # scaled copy of LO on the Activation engine in parallel
                    # with whatever DVE is doing, then add HI on DVE
                    nc.scalar.activation(out=hslc, in_=pl, func=AF.Copy,
                                         scale=1.0 / LSCALE)
                    nc.vector.scalar_tensor_tensor(
                        out=hslc, in0=ph, scalar=1.0, in1=hslc,
                        op0=OP.mult, op1=OP.add)

            def open_banks():
                # psum2h openers emitted up front: the PE runs them during
                # the startup DMA window (it is idle until W1/x pieces land)
                for half in range(2):
                    p2 = psum2h[half]
                    if FP16X3:
                        # full-width opener with s2h, then patch the lo
                        # region to s2l' via (+s2l', -s2h) exact fp16 rows
                        nc.tensor.matmul(p2, lhsT=s2sb[:, 0:A], rhs=ones,
                                         start=True, stop=False,
                                         skip_group_check=True)
                        nc.tensor.matmul(p2[:, CH:], lhsT=s2sb[:, A:2 * A],
                                         rhs=ones[:, :CH], start=False,
                                         stop=False, skip_group_check=True)
                        nc.tensor.matmul(p2[:, CH:], lhsT=s2sb[:, 2 * A:3 * A],
                                         rhs=ones[:, :CH], start=False,
                                         stop=False, skip_group_check=True)
                    else:
                        nc.tensor.matmul(p2[:, :CH], lhsT=s2sb,
                                         rhs=ones32[:, :CH], start=True,
                                         stop=False, skip_group_check=True)

            open_banks()

            def mm2(m, keep_ap, half, stop=False, cs_hi=None, cs_lo=None):
                """Accumulate chunk m of h2 = (s2+b2) - keep @ W2.T into
                psum2h[half] (fp16 path: hi+lo regions of one bank)."""
                p2 = psum2h[half]
                hi = cs_hi if cs_hi is not None else slice(0, CH)
                lo = cs_lo if cs_lo is not None else slice(CH, COLS)
                nc.tensor.matmul(
                    p2[:, hi], lhsT=w2sb[:, m * A:(m + 1) * A],
                    rhs=keep_ap, start=False, stop=(stop and not FP16X3),
                    skip_group_check=True)
                if FP16X3:
                    nc.tensor.matmul(
                        p2[:, lo], lhsT=w2sb[:, (M + m) * A:(M + m + 1) * A],
                        rhs=keep_ap, start=False, stop=stop,
                        skip_group_check=True)

            def h2_evac(csl, half, ps_hi, ps_lo):
                if FP16X3:
                    # Act: h2 = 2^-12 * LO; DVE: h2 += HI (one PSUM src each)
                    nc.scalar.activation(out=h2sb[:, csl],
                                         in_=psum2h[half][:, ps_lo],
                                         func=AF.Copy, scale=1.0 / LSCALE)
                    nc.vector.scalar_tensor_tensor(
                        out=h2sb[:, csl], in0=psum2h[half][:, ps_hi],
                        scalar=1.0, in1=h2sb[:, csl],
                        op0=OP.mult, op1=OP.add)
                else:
                    nc.vector.tensor_copy(h2sb[:, csl], psum2h[half][:, ps_hi])

            L2E = nc.gpsimd if LIF2_ON_POOL else nc.vector
            # layer-2 matmuls for a finished group are emitted mid-way into
            # the NEXT sweep so the PE reaches them after the keep operand
            # is ready (emitting them right between sweeps stalled the PE)
            pending_mm2 = []

            def _flush_pending():
                for fn in pending_mm2:
                    fn()
                pending_mm2.clear()

            # first group is 3-wide: spreads the 13MB x load over a 3x
            # longer PE window (2-wide group-0 sits at ~84% of HBM peak and
            # stalls); 3 chunks x hi/lo = 6 psum banks + 2 layer-2 banks = 8
            GROUPS = [(0, 1, 2), (3, 4), (5, 6)]
            THR = THR1
            for gms in GROUPS:
                nch = len(gms)
                h1g = hpool.tile([128, nch * COLS], F32, tag="h1g")
                phs = [ps1.tile([128, COLS], F32, tag="ps1", name="ph")
                       for _ in gms]
                pls = [ps1.tile([128, COLS], F32, tag="ps1", name="pl")
                       for _ in gms] if FP16X3 else [None] * nch

                def _emit_deferred_x():
                    # sync-queue position: after group-0's W1, before the
                    # k >= 25 matmuls that consume these chunks
                    for out_ap, in_ap in deferred_x:
                        nc.sync.dma_start(out=out_ap, in_=in_ap)
                    deferred_x.clear()

                hooks = {(1, 0): ([_emit_deferred_x] if gms[0] == 0
                                  else [_flush_pending])}
                k_sweep(gms, phs, pls, slice(0, COLS), finely=(gms[0] == 0),
                        hooks=hooks)
                for c, m in enumerate(gms):
                    evac(h1g[:, c * COLS:(c + 1) * COLS], phs[c], pls[c])
                h4 = h1g.rearrange("p (c t b) -> p c b t", c=nch, t=T)
                keepg = kpool.tile([128, nch * COLS], MMDT, tag="keep")
                k4 = keepg.rearrange("p (c t b) -> p c b t", c=nch, t=T)
                memv = mem1v[:, gms[0]:gms[0] + nch, :]
                accg = spool.tile([128, nch * BL], F32, tag="acc")
                accv = accg.rearrange("p (c b) -> p c b", c=nch)
                _lif_steps(nc, memv, accv,
                           lambda t: h4[..., t], lambda t: k4[..., t], THR)

                def _mm2_group(gms=gms, keepg=keepg):
                    for c, m in enumerate(gms):
                        for half in range(2):
                            mm2(m, keepg[:, c * COLS + half * CH:
                                         c * COLS + (half + 1) * CH], half)
                pending_mm2.append(_mm2_group)

            # m = 7 runs in two column (time) phases: while the PE sweeps
            # phase B (t >= TH_A), DVE runs LIF1(m7, phase A) + LIF2(A).
            # Phase B is 14 timesteps = 224 cols, the smallest width that
            # stays matmul-bound (below ~224 cols LDWEIGHTS dominates), so
            # the exposed tail scan is as short as possible.
            m = M - 1
            TH_A = 18
            CA = TH_A * BL
            h1g7 = hpool.tile([128, COLS], F32, tag="h1g")
            keep7 = kpool.tile([128, COLS], MMDT, tag="keep")
            mem7 = mem1v[:, m, :]
            acc7v = spool.tile([128, BL], F32, tag="acc", name="acc7")

            def h_at7(t):
                return h1g7[:, t * BL:(t + 1) * BL]

            def k_at7(t):
                return keep7[:, t * BL:(t + 1) * BL]

            def h2_at(t):
                return h2sb[:, t * BL:(t + 1) * BL]

            def k2_at(t):
                return keep2[:, t * BL:(t + 1) * BL]

            # ---- phase A (t < TH_A) ----
            csA = slice(0, CA)
            phA = ps1.tile([128, CA], F32, tag="ps1", name="ph7")
            plA = (ps1.tile([128, CA], F32, tag="ps1", name="pl7")
                   if FP16X3 else None)
            k_sweep([m], [phA], [plA], csA,
                    hooks={(1, 22): [_flush_pending]})
            evac(h1g7[:, csA], phA, plA)
            _lif_steps(nc, mem7, acc7v, h_at7, k_at7, THR, t_range=range(TH_A))

            def _mm2_a():
                # cols [0, CH) close bank 0; the TH_A-16 timesteps that
                # spill past CH accumulate into bank 1
                mm2(m, keep7[:, 0:CH], 0, stop=True)
                mm2(m, keep7[:, CH:CA], 1,
                    cs_hi=slice(0, CA - CH), cs_lo=slice(CH, CA))
                h2_evac(slice(0, CH), 0, slice(0, CH), slice(CH, COLS))
                h2_evac(slice(CH, CA), 1, slice(0, CA - CH), slice(CH, CA))
                _lif_steps(nc, mem2, acc2, h2_at, k2_at, 1.0,
                           t_range=range(TH_A), eng=L2E)
                # output holds keep2; host computes spk = 1 - keep
                nc.sync.dma_start(out=out_e.ap()[:, csA], in_=keep2[:, csA])
            pending_mm2.append(_mm2_a)

            # ---- phase B (t >= TH_A), pipelined in sub-chunks ----
            csB = slice(CA, COLS)
            phB = ps1.tile([128, COLS - CA], F32, tag="ps1", name="ph7")
            plB = (ps1.tile([128, COLS - CA], F32, tag="ps1", name="pl7")
                   if FP16X3 else None)
            k_sweep([m], [phB], [plB], csB, hooks={(1, 5): [_flush_pending]})
            evac(h1g7[:, csB], phB, plB)
            SUBS = [4, 4, 3, 3] if SUBCHUNK_TAIL else [T - TH_A]
            t0 = TH_A
            for si, sw in enumerate(SUBS):
                t1 = t0 + sw
                _lif_steps(nc, mem7, acc7v, h_at7, k_at7, THR,
                           t_range=range(t0, t1))
                sub = slice(t0 * BL, t1 * BL)            # output columns
                ps_hi = slice(t0 * BL - CH, t1 * BL - CH)
                ps_lo = slice(t0 * BL, t1 * BL)
                mm2(m, keep7[:, sub], 1, stop=(si == len(SUBS) - 1),
                    cs_hi=ps_hi, cs_lo=ps_lo)
                h2_evac(sub, 1, ps_hi, ps_lo)
                _lif_steps(nc, mem2, acc2, h2_at, k2_at, 1.0,
                           t_range=range(t0, t1), eng=L2E)
                nc.sync.dma_start(out=out_e.ap()[:, sub], in_=keep2[:, sub])
                t0 = t1

    nc.compile()
    return nc


def _split16(a):
    """fp32 array -> (hi, lo) fp16 with lo scaled by 2^12."""
    hi = a.astype(np.float16)
    lo = ((a - hi.astype(np.float32)) * LSCALE).astype(np.float16)
    return hi, lo


def _prep_shared(W1, b1, W2, b2):
    W1p = np.zeros((HP, D), np.float32)
    W1p[:H] = W1
    b1p = np.zeros((1, HP), np.float32)
    b1p[0, :H] = b1
    if FP16X3:
        W1p *= WSCALE
        b1p = b1p * WSCALE
    # w1T[m,p,k,j] = W1p[m*128+j, k*128+p]
    w1T = np.ascontiguousarray(
        W1p.reshape(M, 128, KT, 128).transpose(0, 3, 2, 1))
    W2p = np.zeros((A, HP), np.float32)
    W2p[:, :H] = W2
    # w2n[p, m*4+a] = -W2p[a, m*128+p]
    w2n = np.ascontiguousarray((-W2p).reshape(A, M, 128).transpose(2, 1, 0)
                               .reshape(128, M * A))
    s2b2 = (W2p.sum(axis=1, dtype=np.float32) + b2).reshape(1, A)
    s2b2 = np.ascontiguousarray(s2b2.astype(np.float32))
    if FP16X3:
        shared = {}
        w2h, w2l = _split16(w2n)
        shared["w2x"] = np.ascontiguousarray(
            np.concatenate([w2h, w2l], axis=1))
        s2h, s2l = _split16(s2b2)
        shared["s2x"] = np.ascontiguousarray(
            np.concatenate([s2h, s2l, -s2h], axis=1))
        shared["w1h"], shared["w1l"] = _split16(w1T)
        bh, bl = _split16(b1p)
        shared["b1hl"] = np.concatenate([bh, bl], axis=1)
    else:
        shared = {"w2n": w2n, "s2b2": s2b2}
        shared["w1h"] = w1T
        shared["b1hl"] = b1p
    return shared


def _prep_x(x, c):
    # rows t-major: row = t*16 + b
    xs = np.ascontiguousarray(
        x[c * BL:(c + 1) * BL].transpose(1, 0, 2)).reshape(COLS, D)
    xT = np.ascontiguousarray(xs.T)                    # [D, COLS]
    # [128(p), KT, COLS]: xT3[p,k,c] = xT[k*128+p, c]
    xT3 = np.ascontiguousarray(xT.reshape(KT, 128, COLS).transpose(1, 0, 2))
    if FP16X3:
        hi, lo = _split16(xT3)
        return {"xh": hi, "xl": lo}
    return {"xh": xT3}


def kernel(x, W1, b1, W2, b2, _want_results=False):
    x = np.ascontiguousarray(np.asarray(x), np.float32)
    W1 = np.asarray(W1, np.float32)
    b1 = np.asarray(b1, np.float32)
    W2 = np.asarray(W2, np.float32)
    b2 = np.asarray(b2, np.float32)

    with_b1 = bool(np.any(b1))
    key = ("nc", with_b1)
    if key not in _CACHE:
        _CACHE[key] = build(with_b1=with_b1)
    nc = _CACHE[key]

    shared = _prep_shared(W1, b1, W2, b2)
    in_maps = []
    for c in range(NCORES):
        m = dict(shared)
        m.update(_prep_x(x, c))
        in_maps.append(m)

    res = run_bass_kernel_spmd(nc, in_maps, core_ids=list(range(NCORES)))

    out = np.empty((B, T, A), np.float32)
    for c in range(NCORES):
        o = res.results[c]["out"]                      # [A, COLS], col=t*16+b
        # device writes keep2; spk2 = 1 - keep2
        out[c * BL:(c + 1) * BL] = (1.0 - o.T).reshape(T, BL, A).transpose(1, 0, 2)
    if _want_results:
        return out, res
    return out


# revision 14
# speedup vs baseline: 1.0856x; 1.0856x over previous
"""Trainium2 Bass kernel for nn_AdaptiveSNN (B=128, T=32, D=6400, H=1000, A=4).

Strategy (data-parallel over batch, 8 NeuronCores, 16 batch rows each):

  The heavy layer-1 matmul h1[b,t,:] = x[b,t,:] @ W1.T is NOT sequential in t
  (the LIF recurrence only couples the cheap elementwise state update), so per
  core we compute H1 = X_local @ W1.T as one [512, 6400] x [6400, 1024] matmul
  (H padded 1000->1024), laid out transposed: psum banks hold H1.T chunks
  [128 H, 512 cols] with col = t*16 + b (t-major, so per-step LIF slices are
  contiguous 16-element runs and layer-2 column ranges by time are contiguous).

  fp16 hi/lo x3 matmul: fp32 operands are split a = ah + al with ah = fp16(a),
  al = fp16((a - ah) * 2^12); the product needs ah*bh (psum bank HI) and
  ah*bl + al*bh (psum bank LO, uniformly scaled 2^12); al*bl (~2^-24 relative)
  is dropped.  h = HI + 2^-12 * LO then matches an fp32 matmul up to normal
  fp32 rounding (fp16 products are exact in fp32; PSUM accumulates fp32).
  W1 is pre-scaled by 256 so its lo-part stays in fp16 normal range; the LIF
  recurrence is scale-invariant, so mem1 simply runs at 256x with threshold
  256 (exact powers of two).  fp16 streams 1 cycle/row through the PE vs ~6
  effective for fp32.

  - lhsT = W1.T tiles (host pre-transposed), rhs = X.T tiles (host
    pre-transposed), K = D on partitions, 50 k-tiles of 128.
  - m-outer loop over 8 H-chunks; the first LIF group's chunks are
    k-interleaved so the 13MB x load spreads over the first sweep.  W1
    streams on the Sync HWDGE queue, x and constants on the Scalar HWDGE
    queue (two independent FIFOs).
  - x chunks are split into fine pieces so consumers wait per-piece, not for
    a whole 1.3MB transfer (a monolithic chunk 1 cost a 4.9us PE stall).
  - LIF scan: 3 scalar_tensor_tensor ops per step (all hit the DVE 2x_2p
    fast path, ~115ns each, vs tensor_tensor/tensor_scalar at ~205ns):
      acc_t  = beta*mem + h_t                      (mult, add)
      keep_t = (acc_t <= thr) is_ge keep_{t-1}     (is_le, is_ge)  [exact:
               keep_prev=0 -> mem_t=0 -> keep_t=1 = x is_ge 0]
      mem_t  = (acc_t * 1) * keep_{t-1}            (mult, mult)
    keep_t is independent of mem_t, so mm2 unblocks one op earlier.
  - evac h = HI + 2^-12*LO as a single stt reading both psum banks.
  - Layer 2: h2 = spk1 @ W2.T + b2 = (sum(W2)+b2) - keep1 @ W2.T, accumulated
    per chunk as fp16 hi/lo matmuls into one psum bank per time-half (hi sums
    in cols [0,256), 2^12-scaled lo sums in [256,512); one full-width
    start=True opener per bank, emitted during the startup DMA window).
  - The last H-chunk (m7) runs in two column(=time) phases (18/14 steps;
    224 cols is the smallest matmul-bound sweep width); phase B is pipelined
    in sub-chunks: DVE scans a few LIF1 steps, the PE accumulates the mm2
    slice for those columns, DVE runs LIF2 and the output DMA goes out,
    shortening the exposed serial tail.
  - Output is keep2 as [A, 512]; host computes spk2 = 1 - keep2 and
    transposes back.  Each group's layer-2 matmuls are emitted mid-way into
    the next sweep so the PE reaches them after keep is ready.

  (fp32r was measured at ~1e-3 error on HW; with only ~300 output spikes a
  single threshold flip fails the rel-err gate, so only fp32-grade math is
  usable: the fp64 margin analysis shows layer-1 decision margins down to
  6e-6.  This fp16x3 kernel is bit-identical to the jax fp32 reference
  output on the benchmark inputs.)
"""

import sys
import types

import numpy as np

# bass_utils imports antenv.axon_hooks when BASS_TRACE is set; the module is
# absent in some images -- degrade to no tracing instead of crashing.
try:
    import antenv.axon_hooks  # noqa: F401
except ImportError:
    _m = types.ModuleType("antenv.axon_hooks")
    _m.get_axon_ntff_profile_hook = lambda: None
    _m.set_axon_ntff_profile_hook = lambda h: None
    sys.modules["antenv.axon_hooks"] = _m

import concourse.bass as bass
import concourse.tile as tile
from concourse import bacc, mybir
from concourse.bass_utils import run_bass_kernel_spmd

F32 = mybir.dt.float32
F16 = mybir.dt.float16
OP = mybir.AluOpType
AF = mybir.ActivationFunctionType

NCORES = 8
B, T, D, H, A = 128, 32, 6400, 1000, 4
BL = B // NCORES            # 16 local batch
COLS = BL * T               # 512 matmul columns, col = t*16 + b (t-major)
KT = D // 128               # 50 k tiles
HP = 1024                   # padded H
M = HP // 128               # 8 H-chunks
BETA = 1.0 - 0.01

WSCALE = 256.0              # W1 pre-scale (exact power of 2)
LSCALE = 4096.0             # lo-part scale 2^12

# FP16X3 True: hi/lo fp16 3-pass matmul.  False: plain fp32 matmul.
FP16X3 = True

# scheduling feature flags (validated by microbench + HW runs)
STT_SCAN = True             # 3x scalar_tensor_tensor LIF step
# NOTE: a single stt reading both psum banks is ILLEGAL (NCC_IBVF027: only
# one non-scalar input may come from PSUM) -- evac splits across Act + DVE.
EVAC_FUSED = False
# NOTE: Pool rejects TensorScalarPtr outright (NCC_IXCG966) -- LIF2 on DVE.
LIF2_ON_POOL = False
SUBCHUNK_TAIL = True        # pipeline m7 phase-B LIF1/mm2/LIF2 in sub-chunks

XCH = 5                     # x DMA chunks (10 k-tiles each)
XKT = KT // XCH
W1H = 2                     # w1 DMA halves per m-chunk (25 k-tiles each)
W1KT = KT // W1H

_CACHE = {}


def _lif_steps(nc, memv, accv, h_at, k_at, thresh, t_range=None, eng=None):
    """Emit the LIF recurrence for timesteps in t_range (default all).

    memv/accv: [p, ..., b] fp32 SBUF views; h_at(t)/k_at(t) return the
    per-step views.  keep column t holds (mem_t <= thresh) = 1 - spk_t.
    """
    eng = eng if eng is not None else nc.vector
    ts_list = list(t_range if t_range is not None else range(T))
    for t in ts_list:
        skip_mem = (t == T - 1)  # mem after the last step is never read
        if t == 0:
            # mem=0, keep=1: mem <- h_0  (beta*0 + h)
            eng.scalar_tensor_tensor(
                out=memv, in0=memv, scalar=BETA,
                in1=h_at(0), op0=OP.mult, op1=OP.add)
            eng.tensor_scalar(
                out=k_at(0), in0=memv, scalar1=thresh,
                scalar2=None, op0=OP.is_le)
        elif STT_SCAN:
            eng.scalar_tensor_tensor(
                out=accv, in0=memv, scalar=BETA,
                in1=h_at(t), op0=OP.mult, op1=OP.add)
            # keep_t = (acc_t <= thr) is_ge keep_{t-1}; exact because
            # keep_{t-1}=0 forces mem_t=0 which always keeps.
            eng.scalar_tensor_tensor(
                out=k_at(t), in0=accv, scalar=thresh,
                in1=k_at(t - 1), op0=OP.is_le, op1=OP.is_ge)
            if not skip_mem:
                eng.scalar_tensor_tensor(
                    out=memv, in0=accv, scalar=1.0,
                    in1=k_at(t - 1), op0=OP.mult, op1=OP.mult)
        else:
            eng.scalar_tensor_tensor(
                out=accv, in0=memv, scalar=BETA,
                in1=h_at(t), op0=OP.mult, op1=OP.add)
            eng.tensor_tensor(
                out=memv, in0=accv, in1=k_at(t - 1), op=OP.mult)
            eng.tensor_scalar(
                out=k_at(t), in0=memv, scalar1=thresh,
                scalar2=None, op0=OP.is_le)


def build(with_b1=True):
    nc = bacc.Bacc("TRN2", target_bir_lowering=False, debug=False,
                   num_devices=NCORES)

    MMDT = F16 if FP16X3 else F32
    THR1 = 1.0 * WSCALE if FP16X3 else 1.0

    # host layouts (see _prep_shared/_prep_x for the exact packing):
    #   xh/xl [128(p), KT, COLS]      x.T tiles, col = t*16+b, hi/lo fp16
    #   w1h/w1l [M, 128(p), KT, 128]  (256*W1).T tiles, hi/lo fp16
    #   b1hl  [1, 2*HP]               256*b1 hi/lo rows
    #   w2x  [128(p), 2*M*A]          -W2p hi/lo blocks, w2[p, m*4+a]
    #   s2x  [1, 3*A]                 [s2h | s2l' | -s2h], s2 = sum(W2p)+b2
    xh_e = nc.declare_dram_parameter("xh", [128, KT, COLS], MMDT, isOutput=False)
    w1h_e = nc.declare_dram_parameter("w1h", [M, 128, KT, 128], MMDT, isOutput=False)
    b1h_e = nc.declare_dram_parameter("b1hl", [1, (2 * HP if FP16X3 else HP)],
                                      MMDT, isOutput=False)
    if FP16X3:
        xl_e = nc.declare_dram_parameter("xl", [128, KT, COLS], F16, isOutput=False)
        w1l_e = nc.declare_dram_parameter("w1l", [M, 128, KT, 128], F16, isOutput=False)
    if FP16X3:
        # negated W2 in hi/lo fp16 (lo scaled 2^12): [hi | lo] blocks
        w2_e = nc.declare_dram_parameter("w2x", [128, 2 * M * A], F16,
                                         isOutput=False)
        # [s2h | s2l' | -s2h] rows for the dual-region opener
        s2_e = nc.declare_dram_parameter("s2x", [1, 3 * A], F16, isOutput=False)
    else:
        w2_e = nc.declare_dram_parameter("w2n", [128, M * A], F32, isOutput=False)
        s2_e = nc.declare_dram_parameter("s2b2", [1, A], F32, isOutput=False)
    out_e = nc.declare_dram_parameter("out", [A, COLS], F32, isOutput=True)

    with tile.TileContext(nc) as tc:
        with (
            tc.tile_pool(name="const", bufs=1) as cpool,
            tc.tile_pool(name="xsb", bufs=(2 * XCH if FP16X3 else XCH)) as xpool,
            tc.tile_pool(name="w1", bufs=(12 if FP16X3 else 4)) as wpool,
            tc.tile_pool(name="h1g", bufs=2) as hpool,
            tc.tile_pool(name="keep", bufs=2) as kpool,
            tc.tile_pool(name="scratch", bufs=2) as spool,
            tc.tile_pool(name="ps1", bufs=(6 if FP16X3 else 7), space="PSUM") as ps1,
            tc.tile_pool(name="ps2", bufs=1, space="PSUM") as ps2,
        ):
            # Small constants + x go on the Scalar HWDGE queue; W1 streams on
            # the Sync HWDGE queue.  Two independent FIFOs -> W1's first tiles
            # aren't stuck behind 13MB of x.
            ones = cpool.tile([1, COLS], MMDT)
            nc.vector.memset(ones, 1.0)
            ones32 = cpool.tile([1, COLS], F32)
            nc.vector.memset(ones32, 1.0)
            # warm the Activation engine's function table (ACT_TABLE_LOAD is
            # ~1.3us once per func) during the initial DMA wait, so the first
            # evac's scaled-copy isn't delayed by it
            actwarm = cpool.tile([1, 8], F32)
            nc.scalar.activation(out=actwarm, in_=ones32[:, :8], func=AF.Copy,
                                 scale=0.5)
            b1h = b1l = None
            if with_b1:
                b1hl = cpool.tile([1, (2 * HP if FP16X3 else HP)], MMDT)
                nc.scalar.dma_start(out=b1hl, in_=b1h_e.ap())
                b1h = b1hl[:, :HP]
                if FP16X3:
                    b1l = b1hl[:, HP:]

            mem1 = cpool.tile([128, M * BL], F32)
            nc.vector.memset(mem1, 0.0)
            mem1v = mem1.rearrange("p (m b) -> p m b", m=M)
            mem2 = cpool.tile([A, BL], F32)
            nc.vector.memset(mem2, 0.0)
            keep2 = cpool.tile([A, COLS], F32)
            h2sb = cpool.tile([A, COLS], F32)
            acc2 = cpool.tile([A, BL], F32)

            # w2/s2 first on the scalar queue: they are tiny and the
            # psum2h openers (first PE instructions) need s2sb
            W2DT = F16 if FP16X3 else F32
            w2sb = cpool.tile([128, (2 * M * A if FP16X3 else M * A)], W2DT)
            nc.scalar.dma_start(out=w2sb, in_=w2_e.ap())
            s2sb = cpool.tile([1, (3 * A if FP16X3 else A)], W2DT)
            nc.scalar.dma_start(out=s2sb, in_=s2_e.ap())

            # x load in fine pieces: consumers (k-tile matmuls) then wait for
            # their own piece, not a whole chunk.  Chunks 3-4 are DMA'd later
            # on the sync queue (behind group-0's W1): they aren't consumed
            # until ~60% through the first sweep, and deferring them keeps
            # early HBM demand under the limit.
            xparams = [xh_e, xl_e] if FP16X3 else [xh_e]
            xtiles = [[] for _ in xparams]
            deferred_x = []
            sync_x = []
            for xc in range(XCH):
                xts = [xpool.tile([128, XKT * COLS], MMDT, tag="x", name=f"x{xi}")
                       for xi in range(len(xparams))]
                # hi/lo pieces interleaved so the lo stream is never a full
                # chunk behind the hi stream the PE is consuming; fine pieces
                # mean a k-tile's matmul waits for its own piece, not the
                # whole chunk.
                npieces = 10 if xc == 0 else (5 if xc < 3 else 2)
                to_sync = (xc == 2)   # rides the sync queue after w1 half-0
                edges = [xc * XKT + (XKT * p) // npieces
                         for p in range(npieces + 1)]
                for p in range(npieces):
                    k0, k1 = edges[p], edges[p + 1]
                    o0 = (k0 - xc * XKT) * COLS
                    o1 = (k1 - xc * XKT) * COLS
                    for xi, xe in enumerate(xparams):
                        if xc >= 3:
                            deferred_x.append(
                                (xts[xi][:, o0:o1], xe.ap()[:, k0:k1, :]))
                        elif to_sync:
                            sync_x.append(
                                (xts[xi][:, o0:o1], xe.ap()[:, k0:k1, :]))
                        else:
                            nc.scalar.dma_start(
                                out=xts[xi][:, o0:o1], in_=xe.ap()[:, k0:k1, :])
                for xi in range(len(xparams)):
                    xtiles[xi].append(xts[xi])

            def x_rhs(xi, k):
                xt = xtiles[xi][k // XKT]
                o = (k % XKT) * COLS
                return xt[:, o:o + COLS]

            # One psum bank per time-half.  With fp16 W2 the bank holds two
            # regions: hi sums in cols [0,CH), lo sums (2^12-scaled) in
            # [CH,2CH); a single full-width start=True opener avoids the
            # illegal interleaved-starts-on-one-bank pattern.
            psum2h = [ps2.tile([A, COLS], F32, name=f"p2_{h}", tag=f"p2{h}")
                      for h in range(2)]

            wparams = [w1h_e, w1l_e] if FP16X3 else [w1h_e]
            TH = T // 2
            CH = COLS // 2          # column half, t-major: cols [0,CH) = t<TH

            def stream_w1(ms, hf, finely=False):
                """Stream this k-half of W1 for the chunks in ms, pieces
                interleaved across (chunk, hi/lo) so consumers stay in
                lockstep.  Returns {(chunk_idx, dtype_idx): tile}."""
                tiles = {}
                for i in range(len(ms)):
                    for wi in range(len(wparams)):
                        tiles[(i, wi)] = wpool.tile(
                            [128, W1KT * 128], MMDT, tag="w1", name="w1t")
                nq = 5 if finely else 1
                step = W1KT // nq
                for q in range(nq):
                    for i, m in enumerate(ms):
                        for wi, we in enumerate(wparams):
                            nc.sync.dma_start(
                                out=tiles[(i, wi)][:, q * step * 128:
                                                   (q + 1) * step * 128],
                                in_=we.ap()[m, :, hf * W1KT + q * step:
                                            hf * W1KT + (q + 1) * step, :])
                return tiles

            def k_sweep(ms, phs, pls, cs, finely=False, hooks=None):
                """Bias + 50 k-tile matmuls for the chunks in ms over column
                slice cs, k-interleaved across chunks (spreads the DMA demand
                of the first group over twice the PE time).  hooks is a dict
                {(hf, kk): [callables]} fired at that emission point -- used
                to place the previous group's layer-2 matmuls mid-sweep (so
                the PE reaches them well after their keep operand is ready)
                and to defer x DMA emission."""
                ncols = cs.stop - cs.start
                hooks = hooks or {}
                if with_b1:
                    for i, m in enumerate(ms):
                        nc.tensor.matmul(
                            phs[i], lhsT=b1h[:, m * 128:(m + 1) * 128],
                            rhs=ones[:, :ncols], start=True, stop=False)
                        if FP16X3:
                            nc.tensor.matmul(
                                pls[i], lhsT=b1l[:, m * 128:(m + 1) * 128],
                                rhs=ones[:, :ncols], start=True, stop=False)
                for hf in range(W1H):
                    wts = stream_w1(ms, hf, finely=(finely and hf == 0))
                    for kk in range(W1KT):
                        for hook in hooks.get((hf, kk), ()):
                            hook()
                        k = hf * W1KT + kk
                        start = (not with_b1) and k == 0
                        last = (k == KT - 1)
                        sl = slice(kk * 128, (kk + 1) * 128)
                        # hi*hi -> HI bank; hi*lo + lo*hi -> LO bank
                        for i in range(len(ms)):
                            nc.tensor.matmul(
                                phs[i], lhsT=wts[(i, 0)][:, sl],
                                rhs=x_rhs(0, k)[:, cs],
                                start=start, stop=last)
                            if FP16X3:
                                nc.tensor.matmul(
                                    pls[i], lhsT=wts[(i, 0)][:, sl],
                                    rhs=x_rhs(1, k)[:, cs],
                                    start=start, stop=False)
                                nc.tensor.matmul(
                                    pls[i], lhsT=wts[(i, 1)][:, sl],
                                    rhs=x_rhs(0, k)[:, cs],
                                    start=False, stop=last)

            def evac(hslc, ph, pl):
                # h = HI + 2^-12 * LO  (h stays at 256*h1 scale)
                if not FP16X3:
                    nc.vector.tensor_copy(hslc, ph)
                elif EVAC_FUSED:
                    nc.vector.scalar_tensor_tensor(
                        out=hslc, in0=pl, scalar=1.0 / LSCALE, in1=ph,
                        op0=OP.mult, op1=OP.add)
                else:
                    # scaled copy of LO on the Activation engine in parallel
                    # with whatever DVE is doing, then add HI on DVE
                    nc.scalar.activation(out=hslc, in_=pl, func=AF.Copy,
                                         scale=1.0 / LSCALE)
                    nc.vector.scalar_tensor_tensor(
                        out=hslc, in0=ph, scalar=1.0, in1=hslc,
                        op0=OP.mult, op1=OP.add)

            def open_banks():
                # psum2h openers emitted up front: the PE runs them during
                # the startup DMA window (it is idle until W1/x pieces land)
                for half in range(2):
                    p2 = psum2h[half]
                    if FP16X3:
                        # full-width opener with s2h, then patch the lo
                        # region to s2l' via (+s2l', -s2h) exact fp16 rows
                        nc.tensor.matmul(p2, lhsT=s2sb[:, 0:A], rhs=ones,
                                         start=True, stop=False,
                                         skip_group_check=True)
                        nc.tensor.matmul(p2[:, CH:], lhsT=s2sb[:, A:2 * A],
                                         rhs=ones[:, :CH], start=False,
                                         stop=False, skip_group_check=True)
                        nc.tensor.matmul(p2[:, CH:], lhsT=s2sb[:, 2 * A:3 * A],
                                         rhs=ones[:, :CH], start=False,
                                         stop=False, skip_group_check=True)
                    else:
                        nc.tensor.matmul(p2[:, :CH], lhsT=s2sb,
                                         rhs=ones32[:, :CH], start=True,
                                         stop=False, skip_group_check=True)

            open_banks()

            def mm2(m, keep_ap, half, stop=False, cs_hi=None, cs_lo=None):
                """Accumulate chunk m of h2 = (s2+b2) - keep @ W2.T into
                psum2h[half] (fp16 path: hi+lo regions of one bank)."""
                p2 = psum2h[half]
                hi = cs_hi if cs_hi is not None else slice(0, CH)
                lo = cs_lo if cs_lo is not None else slice(CH, COLS)
                nc.tensor.matmul(
                    p2[:, hi], lhsT=w2sb[:, m * A:(m + 1) * A],
                    rhs=keep_ap, start=False, stop=(stop and not FP16X3),
                    skip_group_check=True)
                if FP16X3:
                    nc.tensor.matmul(
                        p2[:, lo], lhsT=w2sb[:, (M + m) * A:(M + m + 1) * A],
                        rhs=keep_ap, start=False, stop=stop,
                        skip_group_check=True)

            def h2_evac(csl, half, ps_hi, ps_lo):
                if FP16X3:
                    # Act: h2 = 2^-12 * LO; DVE: h2 += HI (one PSUM src each)
                    nc.scalar.activation(out=h2sb[:, csl],
                                         in_=psum2h[half][:, ps_lo],
                                         func=AF.Copy, scale=1.0 / LSCALE)
                    nc.vector.scalar_tensor_tensor(
                        out=h2sb[:, csl], in0=psum2h[half][:, ps_hi],
                        scalar=1.0, in1=h2sb[:, csl],
                        op0=OP.mult, op1=OP.add)
                else:
                    nc.vector.tensor_copy(h2sb[:, csl], psum2h[half][:, ps_hi])

            L2E = nc.gpsimd if LIF2_ON_POOL else nc.vector
            # layer-2 matmuls for a finished group are emitted mid-way into
            # the NEXT sweep so the PE reaches them after the keep operand
            # is ready (emitting them right between sweeps stalled the PE)
            pending_mm2 = []

            def _flush_pending():
                for fn in pending_mm2:
                    fn()
                pending_mm2.clear()

            # first group is 3-wide: spreads the 13MB x load over a 3x
            # longer PE window (2-wide group-0 sits at ~84% of HBM peak and
            # stalls); 3 chunks x hi/lo = 6 psum banks + 2 layer-2 banks = 8
            GROUPS = [(0, 1, 2), (3, 4), (5, 6)]
            THR = THR1
            for gms in GROUPS:
                nch = len(gms)
                h1g = hpool.tile([128, nch * COLS], F32, tag="h1g")
                phs = [ps1.tile([128, COLS], F32, tag="ps1", name="ph")
                       for _ in gms]
                pls = [ps1.tile([128, COLS], F32, tag="ps1", name="pl")
                       for _ in gms] if FP16X3 else [None] * nch

                def _emit_deferred_x():
                    # sync-queue position: after group-0's W1, before the
                    # k >= 25 matmuls that consume these chunks
                    for out_ap, in_ap in deferred_x:
                        nc.sync.dma_start(out=out_ap, in_=in_ap)
                    deferred_x.clear()

                def _emit_sync_x():
                    for out_ap, in_ap in sync_x:
                        nc.sync.dma_start(out=out_ap, in_=in_ap)
                    sync_x.clear()

                hooks = ({(0, 1): [_emit_sync_x], (1, 0): [_emit_deferred_x]}
                         if gms[0] == 0 else {(1, 0): [_flush_pending]})
                k_sweep(gms, phs, pls, slice(0, COLS), finely=(gms[0] == 0),
                        hooks=hooks)
                for c, m in enumerate(gms):
                    evac(h1g[:, c * COLS:(c + 1) * COLS], phs[c], pls[c])
                h4 = h1g.rearrange("p (c t b) -> p c b t", c=nch, t=T)
                keepg = kpool.tile([128, nch * COLS], MMDT, tag="keep")
                k4 = keepg.rearrange("p (c t b) -> p c b t", c=nch, t=T)
                memv = mem1v[:, gms[0]:gms[0] + nch, :]
                accg = spool.tile([128, nch * BL], F32, tag="acc")
                accv = accg.rearrange("p (c b) -> p c b", c=nch)
                _lif_steps(nc, memv, accv,
                           lambda t: h4[..., t], lambda t: k4[..., t], THR)

                def _mm2_group(gms=gms, keepg=keepg):
                    for c, m in enumerate(gms):
                        for half in range(2):
                            mm2(m, keepg[:, c * COLS + half * CH:
                                         c * COLS + (half + 1) * CH], half)
                pending_mm2.append(_mm2_group)

            # m = 7 runs in two column (time) phases: while the PE sweeps
            # phase B (t >= TH_A), DVE runs LIF1(m7, phase A) + LIF2(A).
            # Phase B is 14 timesteps = 224 cols, the smallest width that
            # stays matmul-bound (below ~224 cols LDWEIGHTS dominates), so
            # the exposed tail scan is as short as possible.
            m = M - 1
            TH_A = 18
            CA = TH_A * BL
            h1g7 = hpool.tile([128, COLS], F32, tag="h1g")
            keep7 = kpool.tile([128, COLS], MMDT, tag="keep")
            mem7 = mem1v[:, m, :]
            acc7v = spool.tile([128, BL], F32, tag="acc", name="acc7")

            def h_at7(t):
                return h1g7[:, t * BL:(t + 1) * BL]

            def k_at7(t):
                return keep7[:, t * BL:(t + 1) * BL]

            def h2_at(t):
                return h2sb[:, t * BL:(t + 1) * BL]

            def k2_at(t):
                return keep2[:, t * BL:(t + 1) * BL]

            # ---- phase A (t < TH_A) ----
            csA = slice(0, CA)
            phA = ps1.tile([128, CA], F32, tag="ps1", name="ph7")
            plA = (ps1.tile([128, CA], F32, tag="ps1", name="pl7")
                   if FP16X3 else None)
            k_sweep([m], [phA], [plA], csA,
                    hooks={(1, 22): [_flush_pending]})
            evac(h1g7[:, csA], phA, plA)
            _lif_steps(nc, mem7, acc7v, h_at7, k_at7, THR, t_range=range(TH_A))

            def _mm2_a():
                # cols [0, CH) close bank 0; the TH_A-16 timesteps that
                # spill past CH accumulate into bank 1
                mm2(m, keep7[:, 0:CH], 0, stop=True)
                mm2(m, keep7[:, CH:CA], 1,
                    cs_hi=slice(0, CA - CH), cs_lo=slice(CH, CA))
                h2_evac(slice(0, CH), 0, slice(0, CH), slice(CH, COLS))
                h2_evac(slice(CH, CA), 1, slice(0, CA - CH), slice(CH, CA))
                _lif_steps(nc, mem2, acc2, h2_at, k2_at, 1.0,
                           t_range=range(TH_A), eng=L2E)
                # output holds keep2; host computes spk = 1 - keep
                nc.sync.dma_start(out=out_e.ap()[:, csA], in_=keep2[:, csA])
            pending_mm2.append(_mm2_a)

            # ---- phase B (t >= TH_A), pipelined in sub-chunks ----
            csB = slice(CA, COLS)
            phB = ps1.tile([128, COLS - CA], F32, tag="ps1", name="ph7")
            plB = (ps1.tile([128, COLS - CA], F32, tag="ps1", name="pl7")
                   if FP16X3 else None)
            k_sweep([m], [phB], [plB], csB, hooks={(1, 5): [_flush_pending]})
            evac(h1g7[:, csB], phB, plB)
            SUBS = [4, 4, 3, 3] if SUBCHUNK_TAIL else [T - TH_A]
            t0 = TH_A
            for si, sw in enumerate(SUBS):
                t1 = t0 + sw
                _lif_steps(nc, mem7, acc7v, h_at7, k_at7, THR,
                           t_range=range(t0, t1))
                sub = slice(t0 * BL, t1 * BL)            # output columns
                ps_hi = slice(t0 * BL - CH, t1 * BL - CH)
                ps_lo = slice(t0 * BL, t1 * BL)
                mm2(m, keep7[:, sub], 1, stop=(si == len(SUBS) - 1),
                    cs_hi=ps_hi, cs_lo=ps_lo)
                h2_evac(sub, 1, ps_hi, ps_lo)
                _lif_steps(nc, mem2, acc2, h2_at, k2_at, 1.0,
                           t_range=range(t0, t1), eng=L2E)
                nc.sync.dma_start(out=out_e.ap()[:, sub], in_=keep2[:, sub])
                t0 = t1

    nc.compile()
    return nc


def _split16(a):
    """fp32 array -> (hi, lo) fp16 with lo scaled by 2^12."""
    hi = a.astype(np.float16)
    lo = ((a - hi.astype(np.float32)) * LSCALE).astype(np.float16)
    return hi, lo


def _prep_shared(W1, b1, W2, b2):
    W1p = np.zeros((HP, D), np.float32)
    W1p[:H] = W1
    b1p = np.zeros((1, HP), np.float32)
    b1p[0, :H] = b1
    if FP16X3:
        W1p *= WSCALE
        b1p = b1p * WSCALE
    # w1T[m,p,k,j] = W1p[m*128+j, k*128+p]
    w1T = np.ascontiguousarray(
        W1p.reshape(M, 128, KT, 128).transpose(0, 3, 2, 1))
    W2p = np.zeros((A, HP), np.float32)
    W2p[:, :H] = W2
    # w2n[p, m*4+a] = -W2p[a, m*128+p]
    w2n = np.ascontiguousarray((-W2p).reshape(A, M, 128).transpose(2, 1, 0)
                               .reshape(128, M * A))
    s2b2 = (W2p.sum(axis=1, dtype=np.float32) + b2).reshape(1, A)
    s2b2 = np.ascontiguousarray(s2b2.astype(np.float32))
    if FP16X3:
        shared = {}
        w2h, w2l = _split16(w2n)
        shared["w2x"] = np.ascontiguousarray(
            np.concatenate([w2h, w2l], axis=1))
        s2h, s2l = _split16(s2b2)
        shared["s2x"] = np.ascontiguousarray(
            np.concatenate([s2h, s2l, -s2h], axis=1))
        shared["w1h"], shared["w1l"] = _split16(w1T)
        bh, bl = _split16(b1p)
        shared["b1hl"] = np.concatenate([bh, bl], axis=1)
    else:
        shared = {"w2n": w2n, "s2b2": s2b2}
        shared["w1h"] = w1T
        shared["b1hl"] = b1p
    return shared


def _prep_x(x, c):
    # rows t-major: row = t*16 + b
    xs = np.ascontiguousarray(
        x[c * BL:(c + 1) * BL].transpose(1, 0, 2)).reshape(COLS, D)
    xT = np.ascontiguousarray(xs.T)                    # [D, COLS]
    # [128(p), KT, COLS]: xT3[p,k,c] = xT[k*128+p, c]
    xT3 = np.ascontiguousarray(xT.reshape(KT, 128, COLS).transpose(1, 0, 2))
    if FP16X3:
        hi, lo = _split16(xT3)
        return {"xh": hi, "xl": lo}
    return {"xh": xT3}


def kernel(x, W1, b1, W2, b2, _want_results=False):
    x = np.ascontiguousarray(np.asarray(x), np.float32)
    W1 = np.asarray(W1, np.float32)
    b1 = np.asarray(b1, np.float32)
    W2 = np.asarray(W2, np.float32)
    b2 = np.asarray(b2, np.float32)

    with_b1 = bool(np.any(b1))
    key = ("nc", with_b1)
    if key not in _CACHE:
        _CACHE[key] = build(with_b1=with_b1)
    nc = _CACHE[key]

    shared = _prep_shared(W1, b1, W2, b2)
    in_maps = []
    for c in range(NCORES):
        m = dict(shared)
        m.update(_prep_x(x, c))
        in_maps.append(m)

    res = run_bass_kernel_spmd(nc, in_maps, core_ids=list(range(NCORES)))

    out = np.empty((B, T, A), np.float32)
    for c in range(NCORES):
        o = res.results[c]["out"]                      # [A, COLS], col=t*16+b
        # device writes keep2; spk2 = 1 - keep2
        out[c * BL:(c + 1) * BL] = (1.0 - o.T).reshape(T, BL, A).transpose(1, 0, 2)
    if _want_results:
        return out, res
    return out
